# revision 1
# baseline (speedup 1.0000x reference)
"""Trainium2 Bass kernel for nn_Convolution_1176821039249.

Computes out = base_map * mean_k box_k(x) for k in {3,5,7,9,11,13,15} with
replicate padding, on 8 NeuronCores, row-sharded with a 7-row halo.

Algorithm (per core):
  The total 2D kernel K(di,dj) = sum_k 1/(7k^2) * 1[|di|<=k//2] 1[|dj|<=k//2]
  is decomposed over the horizontal "wing" basis
      T_0 = x(center),  T_m(j) = x(j-m) + x(j+m)   (m = 1..7)
  so that  out = sum_{b=0..7} P_b-vertical-band applied to T_b, where
      P_b(d) = sum_{k: k//2 >= max(b,|d|)} 1/(7k^2).
  Wings are one fp16 tensor_tensor add each on DVE (2x mode); the vertical
  pyramid bands are 8 PSUM-accumulated banded matmuls on the PE per tile;
  ACT drains PSUM, GPSIMD multiplies by base_map.
"""

import numpy as np

F16 = np.float16

H = W = 4096
PAD = 7
N_CORES = 8
RPC = H // N_CORES          # 512 output rows per core
TILE_M = 114                # output rows per row tile (128 - 2*PAD)
N_TILES = 5                 # 4 * 114 + 56 = 512
LAST_M = RPC - 4 * TILE_M   # 56
STRIP = 2048                # output cols per strip
N_STRIPS = W // STRIP       # 2
CHUNK = 512                 # matmul N chunk (one PSUM bank of fp32)
KERNEL_SIZES = (3, 5, 7, 9, 11, 13, 15)

_CACHE = {}


def _bands_np() -> np.ndarray:
    """lhsT band matrices, [128, 8*TILE_M] fp16.

    Band b column i row p holds P_b(p - i - 7): the vertical pyramid profile
    applied to wing tensor T_b.
    """
    w = {k: 1.0 / (7.0 * k * k) for k in KERNEL_SIZES}
    P = np.zeros((8, 15), dtype=np.float64)
    for b in range(8):
        for d in range(-7, 8):
            P[b, d + 7] = sum(w[k] for k in KERNEL_SIZES if k // 2 >= max(b, abs(d)))
    M = np.zeros((128, 8 * TILE_M), dtype=np.float64)
    for b in range(8):
        for i in range(TILE_M):
            p_lo = i  # d = p - i - 7 in [-7, 7]; P is indexed at d + 7 = p - i
            for p in range(p_lo, p_lo + 15):
                M[p, b * TILE_M + i] = P[b, p - i]
    return M.astype(F16)


def _build_nc():
    import concourse.bass as bass
    import concourse.mybir as mybir
    import concourse.tile as tile

    dt = mybir.dt
    SHARD_R = RPC + 2 * PAD     # 526
    SHARD_C = W + 2 * PAD       # 4110

    nc = bass.Bass()
    xb_d = nc.declare_dram_parameter("xb", [SHARD_R, SHARD_C], dt.float16, isOutput=False)
    base_d = nc.declare_dram_parameter("base", [RPC, W], dt.float16, isOutput=False)
    bands_d = nc.declare_dram_parameter("bands", [128, 8 * TILE_M], dt.float16, isOutput=False)
    out_d = nc.declare_dram_parameter("out", [RPC, W], dt.float32, isOutput=True)

    with tile.TileContext(nc) as tc:
        with (
            tc.tile_pool(name="const", bufs=1) as constp,
            tc.tile_pool(name="xin", bufs=2) as xpool,
            tc.tile_pool(name="wings", bufs=2) as apool,
            tc.tile_pool(name="io", bufs=2) as iopool,
            tc.tile_pool(name="psum", bufs=2, space="PSUM") as psump,
        ):
            bands_sb = constp.tile([128, 8 * TILE_M], dt.float16, name="bands_sb")
            nc.sync.dma_start(bands_sb[:], bands_d[:])

            for t in range(N_TILES):
                M = TILE_M if t < N_TILES - 1 else LAST_M
                K = M + 2 * PAD
                r0 = t * TILE_M
                # Two loads of the same DRAM rows, offset by one column, so
                # every wing add reads 4B-aligned fp16 pairs (2x_1p DVE mode)
                # in both parities without any on-chip shifted copy.  One
                # dedicated slot per row tile: no slot reuse, so the HWDGE
                # load DMAs carry no sync waits (walrus 1-wait DMA limit).
                xt = xpool.tile([128, SHARD_C], dt.float16, tag="xt", name="xt", bufs=N_TILES)
                x2 = xpool.tile([128, SHARD_C - 1], dt.float16, tag="x2", name="x2", bufs=N_TILES)
                HALF = STRIP + 2 * PAD
                nc.sync.dma_start(xt[:K, :HALF], xb_d[r0:r0 + K, :HALF])
                nc.sync.dma_start(x2[:K, :HALF], xb_d[r0:r0 + K, 1:1 + HALF])
                nc.sync.dma_start(xt[:K, HALF:], xb_d[r0:r0 + K, HALF:])
                nc.sync.dma_start(x2[:K, HALF:], xb_d[r0:r0 + K, 1 + HALF:SHARD_C])
                # fresh slot per tile (like xt): the load carries no sync waits
                bt = iopool.tile([128, W], dt.float16, tag="bt", name="bt", bufs=N_TILES)
                nc.sync.dma_start(bt[:M, :], base_d[r0:r0 + M, :])

                for s in range(N_STRIPS):
                    c0 = s * STRIP
                    def fan(src, start, n, step):
                        # [K, n, STRIP] view: slice i starts at column
                        # start + i*step (overlapping windows; innermost step
                        # stays 1 and starts stay even, so the fp16 2x_1p DVE
                        # mode is preserved)
                        v = src[:K, start:start + STRIP].unsqueeze(1)
                        lst = v.ap
                        lst[1] = (step, n)
                        v.ap = lst
                        return v

                    # Fused wing adds: wings (1,3), (5,7) and (2,4,6) each
                    # collapse into one DVE tensor_tensor via 3D fan views --
                    # their column offsets form stride -2/+2 arithmetic
                    # sequences.  Emitted in the order the PSUM accumulation
                    # chain consumes them (a13 first, a57 last).
                    a13 = apool.tile([128, 2, STRIP], dt.float16, tag="a13", name="a13")
                    nc.vector.tensor_add(a13[:K], fan(xt, c0 + 6, 2, -2), fan(xt, c0 + 8, 2, 2))
                    a246 = apool.tile([128, 3, STRIP], dt.float16, tag="a246", name="a246")
                    nc.vector.tensor_add(a246[:K], fan(x2, c0 + 4, 3, -2), fan(x2, c0 + 8, 3, 2))
                    a57 = apool.tile([128, 2, STRIP], dt.float16, tag="a57", name="a57")
                    nc.vector.tensor_add(a57[:K], fan(xt, c0 + 2, 2, -2), fan(xt, c0 + 12, 2, 2))
                    wings = [a13[:, 0], a246[:, 0], a13[:, 1], a246[:, 1],
                             a57[:, 0], a246[:, 2], a57[:, 1]]  # m = 1..7

                    ps = psump.tile([128, STRIP], dt.float32, tag="ps", name="ps")
                    for b in (0, 1, 3, 2, 4, 6, 5, 7):
                        # center term reads x2 (== xt shifted by 1) so the PE
                        # is not a direct consumer of xt
                        rhs = (x2[:K, c0 + PAD - 1:c0 + PAD - 1 + STRIP]
                               if b == 0 else wings[b - 1][:K, :])
                        lhsT = bands_sb[:K, b * TILE_M:b * TILE_M + M]
                        for c in range(STRIP // CHUNK):
                            nc.tensor.matmul(
                                ps[:M, c * CHUNK:(c + 1) * CHUNK],
                                lhsT,
                                rhs[:, c * CHUNK:(c + 1) * CHUNK],
                                start=(b == 0),
                                stop=(b == 7),
                            )

                    # ACT drains PSUM (Pool cannot read PSUM); the Pool
                    # multiply's extra sync waits are handled by the NoOp
                    # splitter in _split_sync_waits.
                    acc = iopool.tile([128, STRIP], dt.float32, tag="acc", name="acc")
                    if t == N_TILES - 1 and s == N_STRIPS - 1:
                        # tail: chunk drain+mul+store so the store pipelines
                        # with the remaining multiplies instead of serializing
                        # whole-strip stages before the final kernel drain
                        for ci in range(4):
                            cc = slice(ci * CHUNK, (ci + 1) * CHUNK)
                            nc.scalar.copy(acc[:M, cc], ps[:M, cc])
                            nc.vector.tensor_mul(acc[:M, cc], acc[:M, cc],
                                                 bt[:M, c0 + ci * CHUNK:c0 + (ci + 1) * CHUNK])
                            nc.sync.dma_start(out_d[r0:r0 + M, c0 + ci * CHUNK:c0 + (ci + 1) * CHUNK],
                                              acc[:M, cc])
                    else:
                        nc.scalar.copy(acc[:M, :], ps[:M, :])
                        nc.gpsimd.tensor_mul(acc[:M, :], acc[:M, :], bt[:M, c0:c0 + STRIP])
                        nc.sync.dma_start(out_d[r0:r0 + M, c0:c0 + STRIP], acc[:M, :])
    return nc


def _split_sync_waits(nc):
    """Walrus codegen only supports one sync wait per instruction; hoist
    extra waits onto injected NoOps on the instruction's engine (identical
    semantics: the sequencer blocks at the NoOp first, then at the
    instruction).  DMA instructions are issued from their engine's
    sequencer stream, so the same hoisting applies to them.
    """
    import concourse.mybir as mybir

    n_nops = 0
    for fn in nc.m.functions:
        for bb in fn.blocks:
            new = []
            for inst in bb.instructions:
                si = inst.sync_info
                if si is not None and si.on_wait and len(si.on_wait) > 1:
                    waits = list(si.on_wait)
                    hoist, keep = waits[:-1], waits[-1:]
                    for w in hoist:
                        nop = mybir.InstNoOp(name=f"{inst.name}-w{n_nops}", ins=[], outs=[])
                        nop.engine = inst.engine
                        nop.sync_info = mybir.SyncInfo(on_wait=[w], on_update=[])
                        new.append(nop)
                        n_nops += 1
                    if hoist:
                        inst.sync_info = mybir.SyncInfo(
                            on_wait=keep, on_update=list(si.on_update))
                new.append(inst)
            bb.instructions = new
    return n_nops


def _get_nc():
    if "nc" not in _CACHE:
        nc = _build_nc()
        _split_sync_waits(nc)
        _CACHE["nc"] = nc
    return _CACHE["nc"]


def _run(x: np.ndarray, base_map: np.ndarray, trace: bool = False):
    from concourse.bass_utils import run_bass_kernel_spmd

    nc = _get_nc()
    xp = np.pad(np.asarray(x, dtype=np.float32), PAD, mode="edge").astype(F16)
    base_map = np.ascontiguousarray(np.asarray(base_map, dtype=np.float32).astype(F16))
    bands = _bands_np()
    in_maps = []
    for c in range(N_CORES):
        r0 = c * RPC
        in_maps.append({
            "xb": np.ascontiguousarray(xp[r0:r0 + RPC + 2 * PAD]),
            "base": base_map[r0:r0 + RPC],
            "bands": bands,
        })
    res = run_bass_kernel_spmd(nc, in_maps, list(range(N_CORES)), trace=trace)
    out = np.concatenate([res.results[c]["out"] for c in range(N_CORES)], axis=0)
    return out[None, None].astype(np.float32), res


def kernel(x: np.ndarray, base_map: np.ndarray) -> np.ndarray:
    out, _ = _run(x, base_map, trace=False)
    return out



# revision 14
# speedup vs baseline: 1.5650x; 1.5650x over previous
"""Trainium2 Bass kernel for nn_Convolution_1176821039249.

Computes out = base_map * mean_k box_k(x) for k in {3,5,7,9,11,13,15} with
replicate padding, on 8 NeuronCores, row-sharded with a 7-row halo.

Algorithm (per core):
  The total 2D kernel K(di,dj) = sum_k 1/(7k^2) * 1[|di|<=k//2] 1[|dj|<=k//2]
  decomposes over the horizontal "wing" basis
      T_0 = x(center),  T_m(j) = x(j-m) + x(j+m)   (m = 1..7)
  so that  out = sum_{b=0..7} P_b-vertical-band applied to T_b, where
      P_b(d) = sum_{k: k//2 >= max(b,|d|)} 1/(7k^2).

  Band engine split (PE matmul cost is out-width x cycles/row; fp8e4
  DoubleRow runs at 0.5 cycles/row and contracts TWO k-tiles per pass):
    - bands 0..2 (carrying ~91% of kernel mass): fp16 banded matmuls; wing 1
      on DVE (2x_1p), wing 2 on DVE (1x, odd starts), center streams x.
    - bands 3..7: one fp8e4 DoubleRow matmul per band whose two k-tiles are
      the +-m column-shifted views of a host-prepared fp8 copy of x; no
      wing tensors and half the PE cycles.  All band weights are pre-scaled
      by one global S so the fp8 grid lands near the exact coefficients;
      the ACT drain multiplies by 1/S.
  A memset-fed block of dummy matmuls bridges the PE from the preamble to
  the first data-dependent matmul so the p-state ramp (0.65/1.2 GHz for the
  first 3us of busy) is spent while the DMAs land, not on real work.
  ACT drains PSUM to fp16, GPSIMD multiplies by base_map (DVE on the tail
  strip so the final chunks pipeline), and the store + host gather are fp16.
"""

import numpy as np
import ml_dtypes

F16 = np.float16
F8 = ml_dtypes.float8_e4m3

H = W = 4096
PAD = 7
N_CORES = 8
RPC = H // N_CORES          # 512 output rows per core
TILE_M = 114                # output rows per row tile (128 - 2*PAD)
N_TILES = 5                 # 4 * 114 + 56 = 512
LAST_M = RPC - 4 * TILE_M   # 56
STRIP = 2048                # output cols per strip
N_STRIPS = W // STRIP       # 2
CHUNK = 512                 # matmul N chunk (one PSUM bank of fp32)
KERNEL_SIZES = (3, 5, 7, 9, 11, 13, 15)

FP16_BANDS = (0, 1, 2)         # banded fp16 matmuls (wing basis)
FP8_BANDS = (3, 4, 5, 6, 7)    # fp8e4 DoubleRow matmuls on shifted x8
WARMUP = 34                    # dummy 128-col matmuls bridging the PE ramp

_CACHE = {}


def _profiles():
    w = {k: 1.0 / (7.0 * k * k) for k in KERNEL_SIZES}
    P = np.zeros((8, 15), dtype=np.float64)
    for b in range(8):
        for d in range(-7, 8):
            P[b, d + 7] = sum(w[k] for k in KERNEL_SIZES if k // 2 >= max(b, abs(d)))
    return P


def _band_scale(P):
    """Continuous global scale minimizing fp8 quantization error of the
    FP8_BANDS' profiles (deterministic search)."""
    best = None
    for s_exp in np.arange(6.0, 14.0, 0.01):
        s = 2.0 ** s_exp
        var = 0.0
        for b in FP8_BANDS:
            q = (P[b] * s).astype(F16).astype(F8).astype(np.float64) / s
            var += 2.0 * ((q - P[b]) ** 2).sum()
        if best is None or var < best[1]:
            best = (s, var)
    return best[0]


def _bands_np():
    """Returns (bands16, bands8, inv_scale).

    bands16: [128, len(FP16_BANDS)*TILE_M] fp16 lhsT; slot bi column i row p
             holds S * P_b(p - i - 7).
    bands8:  [128, len(FP8_BANDS)*2*128] fp8 lhsT for DoubleRow; band bi's
             two identical k-tile copies of S * P_b(p - i - 7) live in
             128-column blocks (the walrus dual-fp8 Ldweights check requires
             the k-tile step to be even and 16B-aligned, so TILE_M is padded
             to 128).
    """
    P = _profiles()
    S = _band_scale(P)
    M16 = np.zeros((128, len(FP16_BANDS) * TILE_M), dtype=np.float64)
    for bi, b in enumerate(FP16_BANDS):
        for i in range(TILE_M):
            for p in range(i, i + 15):
                M16[p, bi * TILE_M + i] = S * P[b, p - i]
    M8 = np.zeros((128, len(FP8_BANDS), 2, 128), dtype=np.float64)
    for bi, b in enumerate(FP8_BANDS):
        for i in range(TILE_M):
            for p in range(i, i + 15):
                M8[p, bi, :, i] = S * P[b, p - i]
    return (M16.astype(F16), M8.reshape(128, -1).astype(F16).astype(F8),
            float(1.0 / S))


def _build_nc():
    import concourse.bass as bass
    import concourse.mybir as mybir
    import concourse.tile as tile

    dt = mybir.dt
    SHARD_R = RPC + 2 * PAD     # 526
    SHARD_C = W + 2 * PAD       # 4110
    N16 = len(FP16_BANDS) * TILE_M
    N8 = len(FP8_BANDS) * 2 * 128
    _, _, inv_scale = _CACHE["bands"]

    nc = bass.Bass()
    xb_d = nc.declare_dram_parameter("xb", [SHARD_R, SHARD_C], dt.float16, isOutput=False)
    x8_d = nc.declare_dram_parameter("x8", [SHARD_R, SHARD_C], dt.float8e4, isOutput=False)
    base_d = nc.declare_dram_parameter("base", [RPC, W], dt.float16, isOutput=False)
    b16_d = nc.declare_dram_parameter("bands16", [128, N16], dt.float16, isOutput=False)
    b8_d = nc.declare_dram_parameter("bands8", [128, N8], dt.float8e4, isOutput=False)
    out_d = nc.declare_dram_parameter("out", [RPC, W], dt.float16, isOutput=True)

    with tile.TileContext(nc) as tc:
        with (
            tc.tile_pool(name="const", bufs=1) as constp,
            tc.tile_pool(name="xin", bufs=2) as xpool,
            tc.tile_pool(name="wings", bufs=2) as apool,
            tc.tile_pool(name="io", bufs=2) as iopool,
            tc.tile_pool(name="psum", bufs=2, space="PSUM") as psump,
        ):
            # PE warmup: memset scratch, then dummy matmuls that keep the PE
            # continuously busy (p-state ramp) while the first loads land.
            scr = constp.tile([128, 128], dt.float16, name="scr")
            nc.vector.memset(scr[:], 0.25)
            wps = psump.tile([128, 2 * CHUNK], dt.float32, tag="ps", name="wps", bufs=4)
            for i in range(WARMUP):
                nc.tensor.matmul(wps[:128, :128], scr[:, :128], scr[:, :128],
                                 start=True, stop=True)

            b8_sb = constp.tile([128, N8], dt.float8e4, name="b8_sb")
            nc.sync.dma_start(b8_sb[:], b8_d[:])
            b16_sb = constp.tile([128, N16], dt.float16, name="b16_sb")
            nc.sync.dma_start(b16_sb[:], b16_d[:])

            HALF = STRIP + 2 * PAD
            # One dedicated slot per row tile: no slot reuse, so the HWDGE
            # load DMAs carry no sync waits (walrus 1-wait limit).
            xts = [xpool.tile([128, SHARD_C], dt.float16, tag="xt", name="xt", bufs=N_TILES)
                   for _ in range(N_TILES)]
            x8ts = [xpool.tile([128, SHARD_C], dt.float8e4, tag="x8t", name="x8t", bufs=N_TILES)
                    for _ in range(N_TILES)]
            bts = [iopool.tile([128, W], dt.float16, tag="bt", name="bt", bufs=N_TILES)
                   for _ in range(N_TILES)]

            def issue_loads(t):
                M = TILE_M if t < N_TILES - 1 else LAST_M
                K = M + 2 * PAD
                r0 = t * TILE_M
                xt, x8t, bt = xts[t], x8ts[t], bts[t]
                # order: what the tile's first strip consumes first
                nc.sync.dma_start(x8t[:K, :HALF], x8_d[r0:r0 + K, :HALF])
                nc.sync.dma_start(xt[:K, :HALF], xb_d[r0:r0 + K, :HALF])
                nc.sync.dma_start(x8t[:K, HALF:], x8_d[r0:r0 + K, HALF:])
                nc.sync.dma_start(xt[:K, HALF:], xb_d[r0:r0 + K, HALF:])
                nc.sync.dma_start(bt[:M, :STRIP], base_d[r0:r0 + M, :STRIP])
                nc.sync.dma_start(bt[:M, STRIP:], base_d[r0:r0 + M, STRIP:])

            issue_loads(0)
            issue_loads(1)
            for t in range(N_TILES):
                M = TILE_M if t < N_TILES - 1 else LAST_M
                K = M + 2 * PAD
                r0 = t * TILE_M
                xt, x8t, bt = xts[t], x8ts[t], bts[t]
                if t + 2 < N_TILES:
                    # prefetch issue two tiles ahead: the stores' Pool-mult
                    # waits block the SP sequencer for ~2us per strip, and the
                    # DMA engine queue adds ~1.5us; both must be covered or
                    # the next tile's second-half loads land late
                    issue_loads(t + 2)

                for s in range(N_STRIPS):
                    c0 = s * STRIP
                    tail = t == N_TILES - 1

                    def fan(src, start, n, step, width=STRIP):
                        # [K, n, width] view: slice i starts at column
                        # start + i*step (innermost step 1)
                        v = src[:K, start:start + width].unsqueeze(1)
                        lst = v.ap
                        lst[1] = (step, n)
                        v.ap = lst
                        return v

                    # fp16 wings on DVE: wing 1 even starts (2x_1p), wing 2
                    # odd starts (1x).
                    a1 = apool.tile([128, STRIP], dt.float16, tag="a1", name="a1")
                    nc.vector.tensor_add(a1[:K], xt[:K, c0 + 6:c0 + 6 + STRIP],
                                         xt[:K, c0 + 8:c0 + 8 + STRIP])
                    w2 = apool.tile([128, STRIP], dt.float16, tag="w2", name="w2")
                    nc.vector.tensor_add(w2[:K], xt[:K, c0 + 5:c0 + 5 + STRIP],
                                         xt[:K, c0 + 9:c0 + 9 + STRIP])
                    wing16 = {1: a1, 2: w2}

                    order = list(FP8_BANDS) + [0, 1, 2]

                    def mm(ps, b, c, first, last):
                        if b in FP8_BANDS:
                            bi = FP8_BANDS.index(b)
                            lhsT = fan(b8_sb, bi * 2 * 128, 2, 128, width=M)
                            rhs = fan(x8t, c0 + PAD - b + c * CHUNK, 2, 2 * b, width=CHUNK)
                            nc.tensor.matmul(
                                ps[:M, (c % 2) * CHUNK:(c % 2 + 1) * CHUNK],
                                lhsT, rhs, start=first, stop=last,
                                perf_mode=mybir.MatmulPerfMode.DoubleRow,
                            )
                        else:
                            bi = FP16_BANDS.index(b)
                            lhsT = b16_sb[:K, bi * TILE_M:bi * TILE_M + M]
                            rhs = (xt[:K, c0 + PAD:c0 + PAD + STRIP] if b == 0
                                   else wing16[b][:K, :])
                            nc.tensor.matmul(
                                ps[:M, (c % 2) * CHUNK:(c % 2 + 1) * CHUNK],
                                lhsT, rhs[:, c * CHUNK:(c + 1) * CHUNK],
                                start=first, stop=last,
                            )

                    # Per strip, chunks 0+1 accumulate in ps_a[:, :1024] and
                    # chunks 2+3 in ps_b[:, :1024] (two pool slots), each half
                    # drained by one ACT activation (applying the weight
                    # descale) to fp16.  Dependency tracking is tile-granular,
                    # so a drain may only be emitted once its ps tile is fully
                    # written -- with two tiles the first half's
                    # drain+multiply+store overlaps the second half's matmuls,
                    # which empties the pipeline ~2.4us earlier on the final
                    # strip.  GPSIMD does the base_map multiply mid-kernel
                    # (off the critical path); the last tile uses DVE, which
                    # is idle by then and 4x faster per strip.
                    acc = iopool.tile([128, STRIP], dt.float16, tag="acc", name="acc", bufs=4)
                    ps_a = psump.tile([128, 2 * CHUNK], dt.float32, tag="ps", name="ps", bufs=4)
                    for c in (0, 1):
                        for b in order:
                            mm(ps_a, b, c, b == order[0], b == order[-1])
                    ps_b = psump.tile([128, 2 * CHUNK], dt.float32, tag="ps", name="ps", bufs=4)
                    for c in (2, 3):
                        for b in order:
                            mm(ps_b, b, c, b == order[0], b == order[-1])
                    for hi, ph in ((0, ps_a), (1, ps_b)):
                        cc = slice(hi * 2 * CHUNK, (hi + 1) * 2 * CHUNK)
                        nc.scalar.activation(acc[:M, cc], ph[:M, :],
                                             mybir.ActivationFunctionType.Copy,
                                             scale=inv_scale)
                        if tail:
                            nc.vector.tensor_mul(acc[:M, cc], acc[:M, cc],
                                                 bt[:M, c0 + hi * 2 * CHUNK:c0 + (hi + 1) * 2 * CHUNK])
                        else:
                            nc.gpsimd.tensor_mul(acc[:M, cc], acc[:M, cc],
                                                 bt[:M, c0 + hi * 2 * CHUNK:c0 + (hi + 1) * 2 * CHUNK])
                        nc.sync.dma_start(out_d[r0:r0 + M, c0 + hi * 2 * CHUNK:c0 + (hi + 1) * 2 * CHUNK],
                                          acc[:M, cc])
    return nc


def _split_sync_waits(nc):
    """Walrus codegen only supports one sync wait per instruction; hoist
    extra waits onto injected NoOps on the instruction's engine (identical
    semantics: the sequencer blocks at the NoOp first, then at the
    instruction).  DMA instructions are issued from their engine's
    sequencer stream, so the same hoisting applies to them.
    """
    import concourse.mybir as mybir

    n_nops = 0
    for fn in nc.m.functions:
        for bb in fn.blocks:
            new = []
            for inst in bb.instructions:
                si = inst.sync_info
                if si is not None and si.on_wait and len(si.on_wait) > 1:
                    waits = list(si.on_wait)
                    hoist, keep = waits[:-1], waits[-1:]
                    for w in hoist:
                        nop = mybir.InstNoOp(name=f"{inst.name}-w{n_nops}", ins=[], outs=[])
                        nop.engine = inst.engine
                        nop.sync_info = mybir.SyncInfo(on_wait=[w], on_update=[])
                        new.append(nop)
                        n_nops += 1
                    if hoist:
                        inst.sync_info = mybir.SyncInfo(
                            on_wait=keep, on_update=list(si.on_update))
                new.append(inst)
            bb.instructions = new
    return n_nops


def _get_nc():
    if "nc" not in _CACHE:
        if "bands" not in _CACHE:
            _CACHE["bands"] = _bands_np()
        nc = _build_nc()
        _split_sync_waits(nc)
        _CACHE["nc"] = nc
    return _CACHE["nc"]


def _run(x: np.ndarray, base_map: np.ndarray, trace: bool = False):
    from concourse.bass_utils import run_bass_kernel_spmd

    if "bands" not in _CACHE:
        _CACHE["bands"] = _bands_np()
    bands16, bands8, _ = _CACHE["bands"]
    nc = _get_nc()
    xp = np.pad(np.asarray(x, dtype=np.float32), PAD, mode="edge").astype(F16)
    x8 = xp.astype(F8)
    base_map = np.ascontiguousarray(np.asarray(base_map, dtype=np.float32).astype(F16))
    in_maps = []
    for c in range(N_CORES):
        r0 = c * RPC
        in_maps.append({
            "xb": np.ascontiguousarray(xp[r0:r0 + RPC + 2 * PAD]),
            "x8": np.ascontiguousarray(x8[r0:r0 + RPC + 2 * PAD]),
            "base": base_map[r0:r0 + RPC],
            "bands16": bands16,
            "bands8": bands8,
        })
    res = run_bass_kernel_spmd(nc, in_maps, list(range(N_CORES)), trace=trace)
    out = np.concatenate([res.results[c]["out"] for c in range(N_CORES)], axis=0)
    return out[None, None].astype(np.float32), res


def kernel(x: np.ndarray, base_map: np.ndarray) -> np.ndarray:
    out, _ = _run(x, base_map, trace=False)
    return out
